# revision 9
# baseline (speedup 1.0000x reference)
"""Distributed Trainium2 kernel for AnomalyMoE k-NN retrieval.

reference:  q = l2norm(test[L,N,D]); g = l2norm(normal[L,M,D])
            sim[l,n,m] = q . g ; out = (1 - mean_l max_m sim).reshape(1,1,16,16)

Strategy (8 NeuronCores):
- Shard gallery along M (6400 rows/core). Host pre-packs each shard to
  [L, 128, KC, MS] fp8e4m3 so each full-layer DMA moves 51.2KB fully
  contiguous per partition (near-peak HBM efficiency); a small starter
  chunk covers the first two supers so compute starts early.
- Per core: dot[n,m] accumulated on TensorE with fp8 DoubleRow pair-matmuls
  (contraction 256/instruction), supers processed in PAIRS so consecutive
  matmuls share a stationary operand (walrus --enable-ldw-opt dedups the
  redundant LDWEIGHTS).  Gallery row norms via Square + ones-DoubleRow-
  matmul, then ACT Abs_reciprocal_sqrt.  Squares split ACT/DVE
  (KERNEL_SQ_ACT); running per-layer max on DVE or the SDMA CCE datapath
  (KERNEL_MAX_ENG).
- Queries are NOT normalized on the way in: 1/||q_n|| is applied to the
  per-layer maxes at the end (positive per-query scale commutes with max).
- AllReduce(max) over 8 cores per layer (overlapped), or host-side combine
  of per-shard maxes (KERNEL_HOST_COMBINE).
"""

import os
import sys
from concurrent.futures import ThreadPoolExecutor

sys.path.insert(0, "/opt/trn_rl_repo")

import numpy as np
import ml_dtypes

import concourse.bacc as bacc
import concourse.mybir as mybir
import concourse.tile as tile
import concourse.bass_utils as bass_utils
from concourse.bass_utils import run_bass_kernel_spmd

F32 = mybir.dt.float32
BF16 = mybir.dt.bfloat16
AF = mybir.ActivationFunctionType
DR = mybir.MatmulPerfMode.DoubleRow

DT_IN = mybir.dt.float8e4
NP_IN = ml_dtypes.float8_e4m3fn

NCORES = 8
L = 4
D = 1024
N = 256
M_FULL = 51200
MS = M_FULL // NCORES  # 6400 per core
KC = D // 128  # 8 contraction chunks of 128
KP = KC // 2  # 4 DoubleRow pairs
SUPER = 512
STARTER = 1024  # first two supers of layer 0 come via a small early DMA

SKEW_PAIRS = int(os.environ.get("KERNEL_SKEW", "1"))  # pipeline depth in pairs
SQ_ACT = int(os.environ.get("KERNEL_SQ_ACT", "5"))  # k-chunks squared on ACT
MAX_ENG = os.environ.get("KERNEL_MAX_ENG", "dve")  # dve | dma
SPLIT_CC = os.environ.get("KERNEL_SPLIT_CC", "1") == "1"
HOST_COMBINE = os.environ.get("KERNEL_HOST_COMBINE", "0") == "1"
LDWOPT = os.environ.get("KERNEL_LDWOPT", "0") == "1"
BUFS_SQ = int(os.environ.get("KERNEL_BUFS_SQ", "4"))
BUFS_SIM = int(os.environ.get("KERNEL_BUFS_SIM", "4"))
KERNEL_TAG = os.environ.get("KERNEL_TAG", "")
NEG = -3.0e38

_LDW_PATCHED = False


def _patch_ldwopt():
    # walrus skips redundant LDWEIGHTS when consecutive matmuls share a
    # stationary operand; the flag is off in bass_utils' default cmdline.
    global _LDW_PATCHED
    if _LDW_PATCHED or not LDWOPT:
        return
    orig = bass_utils.run_command

    def patched(cmd, **kw):
        cmd = [
            "--enable-ldw-opt=true" if c == "--enable-ldw-opt=false" else c
            for c in cmd
        ]
        return orig(cmd, **kw)

    bass_utils.run_command = patched
    _LDW_PATCHED = True


def build():
    nc = bacc.Bacc("TRN2", target_bir_lowering=False, debug=False, num_devices=NCORES)
    g_ext = nc.dram_tensor("g_t", [L, 128, KC, MS], DT_IN, kind="ExternalInput")
    qt_ext = nc.dram_tensor("q_t", [L, D, N], DT_IN, kind="ExternalInput")
    qn_ext = nc.dram_tensor("q_n", [L, N, D], DT_IN, kind="ExternalInput")
    if HOST_COMBINE:
        lmax_ext = nc.dram_tensor("out_lmax", [128, 2 * L], F32, kind="ExternalOutput")
        invq_ext = nc.dram_tensor("out_invq", [128, 2 * L], F32, kind="ExternalOutput")
        out_ext = cc_in = cc_out = None
    else:
        out_ext = nc.dram_tensor("out", [2, 128], F32, kind="ExternalOutput")
        cc_in = nc.dram_tensor("cc_in", [2 * L, 128], F32)
        cc_out = nc.dram_tensor("cc_out", [2 * L, 128], F32, addr_space="Shared")
        lmax_ext = invq_ext = None

    with tile.TileContext(nc) as tc:
        with (
            tc.tile_pool(name="persist", bufs=1) as pp,
            tc.tile_pool(name="glp", bufs=2) as glpool,
            tc.tile_pool(name="sqp", bufs=BUFS_SQ) as sqpool,
            tc.tile_pool(name="invgp", bufs=4) as invgpool,
            tc.tile_pool(name="simp", bufs=BUFS_SIM) as simpool,
            tc.tile_pool(name="qsqp", bufs=2) as qsqpool,
            tc.tile_pool(name="pm0", bufs=3, space="PSUM") as pm0pool,
            tc.tile_pool(name="pm1", bufs=3, space="PSUM") as pm1pool,
            tc.tile_pool(name="pnorm", bufs=2, space="PSUM") as pnormpool,
        ):
            # ---- persistent tiles ----
            qt_sb = pp.tile([128, L * KC, N], DT_IN, name="qt_sb")
            qn_sb = pp.tile([128, 2 * L, D], DT_IN, name="qn_sb")
            ones_sb = pp.tile([128, 2, 128], DT_IN, name="ones_sb")
            nc.gpsimd.memset(ones_sb[:], 1.0)
            runmax = pp.tile([128, 2 * L, SUPER], BF16, name="runmax")
            nc.gpsimd.memset(runmax[:], NEG)
            starter = pp.tile([128, KC, STARTER], DT_IN, name="starter")
            qss = pp.tile([128, 2 * L], F32, name="qss")
            invq = pp.tile([128, 2 * L], F32, name="invq")
            lmax_sb = pp.tile([128, 2 * L], F32, name="lmax_sb")
            gmax_sb = pp.tile([128, 2 * L], F32, name="gmax_sb")
            smax_sb = pp.tile([128, 2 * L], F32, name="smax_sb")
            res_sb = pp.tile([128, 2], F32, name="res_sb")
            if KERNEL_TAG:
                tag_sb = pp.tile([128, 1], F32, name=f"tag_{KERNEL_TAG}")
                nc.gpsimd.memset(tag_sb[:], 1.0)

            def emit_q_norm_step(step):
                # spread query-norm work through the main loop (ACT bubbles)
                if step == 0:
                    # ACT HWDGE ring: runs concurrent with gallery DMAs (SP ring)
                    nc.scalar.dma_start(
                        qn_sb[:],
                        qn_ext.ap().rearrange("l (c p) d -> p (l c) d", p=128),
                    )
                elif step <= 2 * L:
                    j = step - 1
                    qsq_scr = qsqpool.tile([128, D], BF16, name="qsq_scr")
                    nc.scalar.activation(
                        qsq_scr[:],
                        qn_sb[:, j, :],
                        AF.Square,
                        accum_out=qss[:, j : j + 1],
                    )
                elif step == 2 * L + 1:
                    nc.scalar.activation(invq[:], qss[:], AF.Abs_reciprocal_sqrt)
                    if HOST_COMBINE:
                        nc.sync.dma_start(invq_ext.ap(), invq[:])

            def stage_a(glt, m0, msz):
                # squares: sq[:, k, :msz] = g^2, split ACT / DVE
                sq = sqpool.tile([128, KC, SUPER], DT_IN, name="sq")
                a = SQ_ACT
                if a:
                    nc.scalar.activation(
                        sq[:, :a, :msz], glt[:, :a, m0 : m0 + msz], AF.Square
                    )
                if a < KC:
                    nc.vector.tensor_tensor(
                        out=sq[:, a:, :msz],
                        in0=glt[:, a:, m0 : m0 + msz],
                        in1=glt[:, a:, m0 : m0 + msz],
                        op=mybir.AluOpType.mult,
                    )
                return sq

            def stage_b_pair(items):
                # items: list of (lx, glt, m0, msz, sq), same layer, len 1-2.
                # grouped so consecutive matmuls share stationaries.
                pns = []
                for lx, glt, m0, msz, sq in items:
                    pnorm = pnormpool.tile([128, SUPER], F32, name="pnorm")
                    pns.append(pnorm)
                for j in range(KP):
                    for (lx, glt, m0, msz, sq), pnorm in zip(items, pns):
                        nc.tensor.matmul(
                            pnorm[:, :msz],
                            ones_sb[:],
                            sq[:, 2 * j : 2 * j + 2, :msz],
                            start=(j == 0),
                            stop=(j == KP - 1),
                            perf_mode=DR,
                        )
                invgs = []
                for (lx, glt, m0, msz, sq), pnorm in zip(items, pns):
                    invg = invgpool.tile([128, SUPER], F32, name="invg")
                    nc.scalar.activation(
                        invg[:, :msz], pnorm[:, :msz], AF.Abs_reciprocal_sqrt
                    )
                    invgs.append(invg)

                lx = items[0][0]
                pms = {}
                for cx, pmpool in ((0, pm0pool), (1, pm1pool)):
                    for i, (lx_, glt, m0, msz, sq) in enumerate(items):
                        pms[(i, cx)] = pmpool.tile([128, SUPER], F32, name=f"pm{cx}")
                    for j in range(KP):
                        for i, (lx_, glt, m0, msz, sq) in enumerate(items):
                            nc.tensor.matmul(
                                pms[(i, cx)][:, :msz],
                                qt_sb[
                                    :,
                                    lx * KC + 2 * j : lx * KC + 2 * j + 2,
                                    cx * 128 : (cx + 1) * 128,
                                ],
                                glt[:, 2 * j : 2 * j + 2, m0 : m0 + msz],
                                start=(j == 0),
                                stop=(j == KP - 1),
                                perf_mode=DR,
                            )
                for i, (lx_, glt, m0, msz, sq) in enumerate(items):
                    for cx in range(2):
                        pm = pms[(i, cx)]
                        sim = simpool.tile([128, SUPER], BF16, name="sim")
                        nc.vector.tensor_tensor(
                            out=sim[:, :msz],
                            in0=pm[:, :msz],
                            in1=invgs[i][:, :msz],
                            op=mybir.AluOpType.mult,
                        )
                        j2 = lx * 2 + cx
                        if MAX_ENG == "dma":
                            nc.gpsimd.dma_start(
                                out=runmax[:, j2, :msz],
                                in_=sim[:, :msz],
                                accum_op=mybir.AluOpType.max,
                            )
                        else:
                            nc.vector.tensor_tensor(
                                out=runmax[:, j2, :msz],
                                in0=runmax[:, j2, :msz],
                                in1=sim[:, :msz],
                                op=mybir.AluOpType.max,
                            )

            def layer_done(lx):
                for cx in range(2):
                    j = lx * 2 + cx
                    nc.vector.reduce_max(
                        lmax_sb[:, j : j + 1],
                        runmax[:, j, :],
                        axis=mybir.AxisListType.X,
                    )
                if HOST_COMBINE:
                    nc.sync.dma_start(
                        lmax_ext.ap()[:, 2 * lx : 2 * lx + 2],
                        lmax_sb[:, 2 * lx : 2 * lx + 2],
                    )
                elif SPLIT_CC:
                    nc.sync.dma_start(
                        cc_in.ap()[2 * lx : 2 * lx + 2, :].rearrange("c p -> p c"),
                        lmax_sb[:, 2 * lx : 2 * lx + 2],
                    )
                    nc.gpsimd.collective_compute(
                        "AllReduce",
                        mybir.AluOpType.max,
                        replica_groups=[list(range(NCORES))],
                        ins=[cc_in.ap()[2 * lx : 2 * lx + 2, :].opt()],
                        outs=[cc_out.ap()[2 * lx : 2 * lx + 2, :].opt()],
                    )

            # ---- DMAs: starter chunk, then full contiguous layers ----
            big = {}

            def emit_big_dma(lx):
                m_lo = STARTER if lx == 0 else 0
                w = MS - m_lo
                glt = glpool.tile([128, KC, MS], DT_IN, name="gl")
                if lx == 0:
                    # split so early supers unblock before the full layer lands
                    h = w // 2 // SUPER * SUPER
                    nc.sync.dma_start(
                        glt[:, :, :h], g_ext.ap()[lx][:, :, m_lo : m_lo + h]
                    )
                    nc.sync.dma_start(
                        glt[:, :, h:w], g_ext.ap()[lx][:, :, m_lo + h : MS]
                    )
                else:
                    nc.sync.dma_start(
                        glt[:, :, :w], g_ext.ap()[lx][:, :, m_lo:MS]
                    )
                big[lx] = glt

            nc.sync.dma_start(starter[:], g_ext.ap()[0][:, :, :STARTER])
            nc.scalar.dma_start(
                qt_sb[:], qt_ext.ap().rearrange("l (k p) n -> p (l k) n", p=128)
            )
            emit_q_norm_step(0)  # qn DMA
            emit_big_dma(0)
            emit_big_dma(1)

            # work items: (lx, tile_kind, m0_in_tile, msz)
            def layer_supers(lx):
                out = []
                if lx == 0:
                    for m0 in range(0, STARTER, SUPER):
                        out.append((lx, "S", m0, SUPER))
                    base = STARTER
                else:
                    base = 0
                w = MS - base
                for m0 in range(0, w, SUPER):
                    out.append((lx, "B", m0, min(SUPER, w - m0)))
                return out

            pending = []
            done_lx = 0
            qn_step = 1
            sidx = 0
            for lx in range(L):
                for it in layer_supers(lx):
                    lx_, kind, m0, msz = it
                    if kind == "S":
                        glt = starter
                    else:
                        glt = big[lx_]
                    pending.append((lx_, glt, m0, msz, stage_a(glt, m0, msz)))
                    if sidx >= 2 and sidx % 2 == 0 and qn_step <= 2 * L + 1:
                        emit_q_norm_step(qn_step)
                        qn_step += 1
                    # pop a pair when we have SKEW_PAIRS*2 + 2 items pending
                    while len(pending) > 2 * SKEW_PAIRS + 1:
                        take = 2 if (
                            len(pending) >= 2 and pending[0][0] == pending[1][0]
                        ) else 1
                        batch, pending = pending[:take], pending[take:]
                        stage_b_pair(batch)
                        nxt = pending[0][0] if pending else L
                        while done_lx < nxt:
                            layer_done(done_lx)
                            done_lx += 1
                    sidx += 1
                if lx + 2 < L:
                    emit_big_dma(lx + 2)
            while pending:
                take = 2 if (
                    len(pending) >= 2 and pending[0][0] == pending[1][0]
                ) else 1
                batch, pending = pending[:take], pending[take:]
                stage_b_pair(batch)
                nxt = pending[0][0] if pending else L
                while done_lx < nxt:
                    layer_done(done_lx)
                    done_lx += 1
            while qn_step <= 2 * L + 1:
                emit_q_norm_step(qn_step)
                qn_step += 1

            if not HOST_COMBINE:
                if not SPLIT_CC:
                    nc.sync.dma_start(
                        cc_in.ap().rearrange("c p -> p c"), lmax_sb[:]
                    )
                    nc.gpsimd.collective_compute(
                        "AllReduce",
                        mybir.AluOpType.max,
                        replica_groups=[list(range(NCORES))],
                        ins=[cc_in.ap().opt()],
                        outs=[cc_out.ap().opt()],
                    )
                nc.sync.dma_start(
                    gmax_sb[:], cc_out.ap().rearrange("c p -> p c")
                )

                # ---- scale by 1/||q||, mean over layers, 1 - x ----
                nc.vector.tensor_tensor(
                    out=smax_sb[:],
                    in0=gmax_sb[:],
                    in1=invq[:],
                    op=mybir.AluOpType.mult,
                )
                for cx in range(2):
                    ssum = pp.tile([128, 1], F32, name=f"ssum{cx}")
                    nc.vector.reduce_sum(
                        ssum[:],
                        smax_sb[:, cx : 2 * L : 2],
                        axis=mybir.AxisListType.X,
                    )
                    nc.scalar.activation(
                        res_sb[:, cx : cx + 1],
                        ssum[:],
                        AF.Copy,
                        bias=1.0,
                        scale=-1.0 / L,
                    )
                for cx in range(2):
                    nc.sync.dma_start(
                        out_ext.ap()[cx : cx + 1, :].rearrange("c p -> p c"),
                        res_sb[:, cx : cx + 1],
                    )

    nc.compile()
    return nc


_NC_CACHE = None


def _get_nc():
    global _NC_CACHE
    if _NC_CACHE is None:
        _patch_ldwopt()
        _NC_CACHE = build()
    return _NC_CACHE


def _prep_shard(g_lp, c):
    # [L, MS, D] slice -> [L, 128, KC, MS] (contiguous partition lines)
    sl = g_lp[:, c * MS : (c + 1) * MS, :]  # [L, MS, D]
    # d = k*128 + p  ->  [L, MS, KC, 128] -> [L, 128, KC, MS]
    return np.ascontiguousarray(
        sl.reshape(L, MS, KC, 128).transpose(0, 3, 2, 1)
    )


def _prep_inputs(test_patch_tokens, normal_patch_tokens):
    q = np.asarray(test_patch_tokens, dtype=np.float32)
    g = np.asarray(normal_patch_tokens, dtype=np.float32)
    qn_lp = q.astype(NP_IN)  # [L, N, D]
    qt_lp = np.ascontiguousarray(qn_lp.transpose(0, 2, 1))  # [L, D, N]
    g_lp = g.astype(NP_IN)  # [L, M, D]
    with ThreadPoolExecutor(NCORES) as ex:
        shards = list(ex.map(lambda c: _prep_shard(g_lp, c), range(NCORES)))
    return [
        {"g_t": shards[c], "q_t": qt_lp, "q_n": qn_lp} for c in range(NCORES)
    ]


def kernel(test_patch_tokens: np.ndarray, normal_patch_tokens: np.ndarray):
    in_maps = _prep_inputs(test_patch_tokens, normal_patch_tokens)
    nc = _get_nc()
    results = run_bass_kernel_spmd(nc, in_maps, core_ids=list(range(NCORES))).results
    if HOST_COMBINE:
        # gather/unshard: global max over the 8 gallery shards, then the
        # tiny epilogue (1/||q|| scale, mean over layers, 1-x)
        lmax = np.max(
            np.stack([results[c]["out_lmax"] for c in range(NCORES)]), axis=0
        )  # [128, 2L]: column j = layer*2 + chunk
        invq = results[0]["out_invq"]  # identical on all cores
        smax = lmax * invq
        test_sim = smax.reshape(128, L, 2).mean(axis=1)  # [128(p), 2(chunk)]
        out = 1.0 - test_sim.T.reshape(N)  # n = chunk*128 + p
        return out.astype(np.float32).reshape(1, 1, 16, 16)
    out = results[0]["out"].astype(np.float32).reshape(1, 1, 16, 16)
    return out


# revision 13
# speedup vs baseline: 1.1572x; 1.1572x over previous
"""Distributed Trainium2 kernel for AnomalyMoE k-NN retrieval.

reference:  q = l2norm(test[L,N,D]); g = l2norm(normal[L,M,D])
            sim[l,n,m] = q . g ; out = (1 - mean_l max_m sim).reshape(1,1,16,16)

Strategy (8 NeuronCores):
- Shard gallery along M (6400 rows/core). Host pre-packs each shard to
  [L, 128, KC, MS] fp8e4m3 so each full-layer DMA moves 51.2KB fully
  contiguous per partition (near-peak HBM efficiency); a small starter
  chunk covers the first two supers so compute starts early.
- Per core: dot[n,m] accumulated on TensorE with fp8 DoubleRow pair-matmuls
  (contraction 256/instruction), supers processed in PAIRS so consecutive
  matmuls share a stationary operand (walrus --enable-ldw-opt dedups the
  redundant LDWEIGHTS).  Gallery row norms via Square + ones-DoubleRow-
  matmul, then ACT Abs_reciprocal_sqrt.  Squares split ACT/DVE
  (KERNEL_SQ_ACT); running per-layer max on DVE or the SDMA CCE datapath
  (KERNEL_MAX_ENG).
- Queries are NOT normalized on the way in: 1/||q_n|| is applied to the
  per-layer maxes at the end (positive per-query scale commutes with max).
- AllReduce(max) over 8 cores per layer (overlapped), or host-side combine
  of per-shard maxes (KERNEL_HOST_COMBINE).
"""

import os
import sys
from concurrent.futures import ThreadPoolExecutor

sys.path.insert(0, "/opt/trn_rl_repo")

import numpy as np
import ml_dtypes

import concourse.bacc as bacc
import concourse.mybir as mybir
import concourse.tile as tile
import concourse.bass_utils as bass_utils
from concourse.bass_utils import run_bass_kernel_spmd

F32 = mybir.dt.float32
BF16 = mybir.dt.bfloat16
AF = mybir.ActivationFunctionType
DR = mybir.MatmulPerfMode.DoubleRow

DT_IN = mybir.dt.float8e4
NP_IN = ml_dtypes.float8_e4m3fn

NCORES = 8
L = 4
D = 1024
N = 256
M_FULL = 51200
MS = M_FULL // NCORES  # 6400 per core
KC = D // 128  # 8 contraction chunks of 128
KP = KC // 2  # 4 DoubleRow pairs
SUPER = 512
STARTER = 1024  # first two supers of layer 0 come via a small early DMA

SKEW_PAIRS = int(os.environ.get("KERNEL_SKEW", "1"))  # pipeline depth in pairs
SQ_ACT = int(os.environ.get("KERNEL_SQ_ACT", "5"))  # k-chunks squared on ACT
MAX_ENG = os.environ.get("KERNEL_MAX_ENG", "dve")  # dve | dma
SPLIT_CC = os.environ.get("KERNEL_SPLIT_CC", "1") == "1"
HOST_COMBINE = os.environ.get("KERNEL_HOST_COMBINE", "0") == "1"
LDWOPT = os.environ.get("KERNEL_LDWOPT", "0") == "1"
BUFS_SQ = int(os.environ.get("KERNEL_BUFS_SQ", "4"))
BUFS_SIM = int(os.environ.get("KERNEL_BUFS_SIM", "4"))
KERNEL_TAG = os.environ.get("KERNEL_TAG", "")
NEG = -3.0e38

_LDW_PATCHED = False


def _patch_ldwopt():
    # walrus skips redundant LDWEIGHTS when consecutive matmuls share a
    # stationary operand; the flag is off in bass_utils' default cmdline.
    global _LDW_PATCHED
    if _LDW_PATCHED or not LDWOPT:
        return
    orig = bass_utils.run_command

    def patched(cmd, **kw):
        cmd = [
            "--enable-ldw-opt=true" if c == "--enable-ldw-opt=false" else c
            for c in cmd
        ]
        return orig(cmd, **kw)

    bass_utils.run_command = patched
    _LDW_PATCHED = True


def build():
    nc = bacc.Bacc("TRN2", target_bir_lowering=False, debug=False, num_devices=NCORES)
    g_ext = nc.dram_tensor("g_t", [L, 128, KC, MS], DT_IN, kind="ExternalInput")
    qt_ext = nc.dram_tensor("q_t", [128, L * KC, N], DT_IN, kind="ExternalInput")
    qn_ext = nc.dram_tensor("q_n", [128, 2 * L, D], DT_IN, kind="ExternalInput")
    if HOST_COMBINE:
        lmax_ext = nc.dram_tensor("out_lmax", [128, 2 * L], F32, kind="ExternalOutput")
        invq_ext = nc.dram_tensor("out_invq", [128, 2 * L], F32, kind="ExternalOutput")
        out_ext = cc_in = cc_out = None
    else:
        out_ext = nc.dram_tensor("out", [2, 128], F32, kind="ExternalOutput")
        cc_in = nc.dram_tensor("cc_in", [2 * L, 128], F32)
        cc_out = nc.dram_tensor("cc_out", [2 * L, 128], F32, addr_space="Shared")
        lmax_ext = invq_ext = None

    with tile.TileContext(nc) as tc:
        with (
            tc.tile_pool(name="persist", bufs=1) as pp,
            tc.tile_pool(name="glp", bufs=2) as glpool,
            tc.tile_pool(name="sqp", bufs=BUFS_SQ) as sqpool,
            tc.tile_pool(name="invgp", bufs=4) as invgpool,
            tc.tile_pool(name="simp", bufs=BUFS_SIM) as simpool,
            tc.tile_pool(name="qsqp", bufs=2) as qsqpool,
            tc.tile_pool(name="pm0", bufs=3, space="PSUM") as pm0pool,
            tc.tile_pool(name="pm1", bufs=3, space="PSUM") as pm1pool,
            tc.tile_pool(name="pnorm", bufs=2, space="PSUM") as pnormpool,
        ):
            # ---- persistent tiles ----
            qt_sb = pp.tile([128, L * KC, N], DT_IN, name="qt_sb")
            qn_sb = pp.tile([128, 2 * L, D], DT_IN, name="qn_sb")
            ones_sb = pp.tile([128, 2, 128], DT_IN, name="ones_sb")
            nc.gpsimd.memset(ones_sb[:], 1.0)
            runmax = pp.tile([128, 2 * L, SUPER], BF16, name="runmax")
            nc.gpsimd.memset(runmax[:], NEG)
            starter = pp.tile([128, KC, STARTER], DT_IN, name="starter")
            qss = pp.tile([128, 2 * L], F32, name="qss")
            invq = pp.tile([128, 2 * L], F32, name="invq")
            lmax_sb = pp.tile([128, 2 * L], F32, name="lmax_sb")
            gmax_sb = pp.tile([128, 2 * L], F32, name="gmax_sb")
            smax_sb = pp.tile([128, 2 * L], F32, name="smax_sb")
            res_sb = pp.tile([128, 2], F32, name="res_sb")
            if KERNEL_TAG:
                tag_sb = pp.tile([128, 1], F32, name=f"tag_{KERNEL_TAG}")
                nc.gpsimd.memset(tag_sb[:], 1.0)

            def emit_q_norm_step(step):
                # spread query-norm work through the main loop (ACT bubbles)
                if step == 0:
                    nc.sync.dma_start(qn_sb[:], qn_ext.ap())
                elif step <= 2 * L:
                    j = step - 1
                    qsq_scr = qsqpool.tile([128, D], BF16, name="qsq_scr")
                    nc.scalar.activation(
                        qsq_scr[:],
                        qn_sb[:, j, :],
                        AF.Square,
                        accum_out=qss[:, j : j + 1],
                    )
                elif step == 2 * L + 1:
                    nc.scalar.activation(invq[:], qss[:], AF.Abs_reciprocal_sqrt)
                    if HOST_COMBINE:
                        nc.sync.dma_start(invq_ext.ap(), invq[:])

            def stage_a(glt, m0, msz):
                # squares: sq[:, k, :msz] = g^2, split ACT / DVE
                sq = sqpool.tile([128, KC, SUPER], DT_IN, name="sq")
                a = SQ_ACT
                if a:
                    nc.scalar.activation(
                        sq[:, :a, :msz], glt[:, :a, m0 : m0 + msz], AF.Square
                    )
                if a < KC:
                    nc.vector.tensor_tensor(
                        out=sq[:, a:, :msz],
                        in0=glt[:, a:, m0 : m0 + msz],
                        in1=glt[:, a:, m0 : m0 + msz],
                        op=mybir.AluOpType.mult,
                    )
                return sq

            def stage_b_pair(items):
                # items: list of (lx, glt, m0, msz, sq), same layer, len 1-2.
                # grouped so consecutive matmuls share stationaries.
                pns = []
                for lx, glt, m0, msz, sq in items:
                    pnorm = pnormpool.tile([128, SUPER], F32, name="pnorm")
                    pns.append(pnorm)
                for j in range(KP):
                    for (lx, glt, m0, msz, sq), pnorm in zip(items, pns):
                        nc.tensor.matmul(
                            pnorm[:, :msz],
                            ones_sb[:],
                            sq[:, 2 * j : 2 * j + 2, :msz],
                            start=(j == 0),
                            stop=(j == KP - 1),
                            perf_mode=DR,
                        )
                invgs = []
                for (lx, glt, m0, msz, sq), pnorm in zip(items, pns):
                    invg = invgpool.tile([128, SUPER], F32, name="invg")
                    nc.scalar.activation(
                        invg[:, :msz], pnorm[:, :msz], AF.Abs_reciprocal_sqrt
                    )
                    invgs.append(invg)

                lx = items[0][0]
                pms = {}
                for cx, pmpool in ((0, pm0pool), (1, pm1pool)):
                    for i, (lx_, glt, m0, msz, sq) in enumerate(items):
                        pms[(i, cx)] = pmpool.tile([128, SUPER], F32, name=f"pm{cx}")
                    for j in range(KP):
                        for i, (lx_, glt, m0, msz, sq) in enumerate(items):
                            nc.tensor.matmul(
                                pms[(i, cx)][:, :msz],
                                qt_sb[
                                    :,
                                    lx * KC + 2 * j : lx * KC + 2 * j + 2,
                                    cx * 128 : (cx + 1) * 128,
                                ],
                                glt[:, 2 * j : 2 * j + 2, m0 : m0 + msz],
                                start=(j == 0),
                                stop=(j == KP - 1),
                                perf_mode=DR,
                            )
                for i, (lx_, glt, m0, msz, sq) in enumerate(items):
                    for cx in range(2):
                        pm = pms[(i, cx)]
                        sim = simpool.tile([128, SUPER], BF16, name="sim")
                        nc.vector.tensor_tensor(
                            out=sim[:, :msz],
                            in0=pm[:, :msz],
                            in1=invgs[i][:, :msz],
                            op=mybir.AluOpType.mult,
                        )
                        j2 = lx * 2 + cx
                        if MAX_ENG == "dma":
                            nc.gpsimd.dma_start(
                                out=runmax[:, j2, :msz],
                                in_=sim[:, :msz],
                                accum_op=mybir.AluOpType.max,
                            )
                        else:
                            nc.vector.tensor_tensor(
                                out=runmax[:, j2, :msz],
                                in0=runmax[:, j2, :msz],
                                in1=sim[:, :msz],
                                op=mybir.AluOpType.max,
                            )

            def layer_done(lx):
                for cx in range(2):
                    j = lx * 2 + cx
                    nc.vector.reduce_max(
                        lmax_sb[:, j : j + 1],
                        runmax[:, j, :],
                        axis=mybir.AxisListType.X,
                    )
                if HOST_COMBINE:
                    nc.sync.dma_start(
                        lmax_ext.ap()[:, 2 * lx : 2 * lx + 2],
                        lmax_sb[:, 2 * lx : 2 * lx + 2],
                    )
                elif SPLIT_CC:
                    nc.sync.dma_start(
                        cc_in.ap()[2 * lx : 2 * lx + 2, :].rearrange("c p -> p c"),
                        lmax_sb[:, 2 * lx : 2 * lx + 2],
                    )
                    nc.gpsimd.collective_compute(
                        "AllReduce",
                        mybir.AluOpType.max,
                        replica_groups=[list(range(NCORES))],
                        ins=[cc_in.ap()[2 * lx : 2 * lx + 2, :].opt()],
                        outs=[cc_out.ap()[2 * lx : 2 * lx + 2, :].opt()],
                    )

            # ---- DMAs: starter chunk, then full contiguous layers ----
            big = {}

            def emit_big_dma(lx):
                m_lo = STARTER if lx == 0 else 0
                w = MS - m_lo
                glt = glpool.tile([128, KC, MS], DT_IN, name="gl")
                if lx == 0:
                    # split so early supers unblock before the full layer lands
                    h = w // 2 // SUPER * SUPER
                    nc.sync.dma_start(
                        glt[:, :, :h], g_ext.ap()[lx][:, :, m_lo : m_lo + h]
                    )
                    nc.sync.dma_start(
                        glt[:, :, h:w], g_ext.ap()[lx][:, :, m_lo + h : MS]
                    )
                else:
                    nc.sync.dma_start(
                        glt[:, :, :w], g_ext.ap()[lx][:, :, m_lo:MS]
                    )
                big[lx] = glt

            nc.sync.dma_start(starter[:], g_ext.ap()[0][:, :, :STARTER])
            nc.sync.dma_start(qt_sb[:], qt_ext.ap())
            emit_q_norm_step(0)  # qn DMA
            emit_big_dma(0)
            emit_big_dma(1)

            # work items: (lx, tile_kind, m0_in_tile, msz)
            def layer_supers(lx):
                out = []
                if lx == 0:
                    for m0 in range(0, STARTER, SUPER):
                        out.append((lx, "S", m0, SUPER))
                    base = STARTER
                else:
                    base = 0
                w = MS - base
                for m0 in range(0, w, SUPER):
                    out.append((lx, "B", m0, min(SUPER, w - m0)))
                return out

            pending = []
            done_lx = 0
            qn_step = 1
            sidx = 0
            for lx in range(L):
                for it in layer_supers(lx):
                    lx_, kind, m0, msz = it
                    if kind == "S":
                        glt = starter
                    else:
                        glt = big[lx_]
                    pending.append((lx_, glt, m0, msz, stage_a(glt, m0, msz)))
                    if sidx >= 2 and sidx % 2 == 0 and qn_step <= 2 * L + 1:
                        emit_q_norm_step(qn_step)
                        qn_step += 1
                    # pop a pair when we have SKEW_PAIRS*2 + 2 items pending
                    while len(pending) > 2 * SKEW_PAIRS + 1:
                        take = 2 if (
                            len(pending) >= 2 and pending[0][0] == pending[1][0]
                        ) else 1
                        batch, pending = pending[:take], pending[take:]
                        stage_b_pair(batch)
                        nxt = pending[0][0] if pending else L
                        while done_lx < nxt:
                            layer_done(done_lx)
                            done_lx += 1
                    sidx += 1
                if lx + 2 < L:
                    emit_big_dma(lx + 2)
            while pending:
                take = 2 if (
                    len(pending) >= 2 and pending[0][0] == pending[1][0]
                ) else 1
                batch, pending = pending[:take], pending[take:]
                stage_b_pair(batch)
                nxt = pending[0][0] if pending else L
                while done_lx < nxt:
                    layer_done(done_lx)
                    done_lx += 1
            while qn_step <= 2 * L + 1:
                emit_q_norm_step(qn_step)
                qn_step += 1

            if not HOST_COMBINE:
                if not SPLIT_CC:
                    nc.sync.dma_start(
                        cc_in.ap().rearrange("c p -> p c"), lmax_sb[:]
                    )
                    nc.gpsimd.collective_compute(
                        "AllReduce",
                        mybir.AluOpType.max,
                        replica_groups=[list(range(NCORES))],
                        ins=[cc_in.ap().opt()],
                        outs=[cc_out.ap().opt()],
                    )
                nc.sync.dma_start(
                    gmax_sb[:], cc_out.ap().rearrange("c p -> p c")
                )

                # ---- scale by 1/||q||, mean over layers, 1 - x ----
                nc.vector.tensor_tensor(
                    out=smax_sb[:],
                    in0=gmax_sb[:],
                    in1=invq[:],
                    op=mybir.AluOpType.mult,
                )
                for cx in range(2):
                    ssum = pp.tile([128, 1], F32, name=f"ssum{cx}")
                    nc.vector.reduce_sum(
                        ssum[:],
                        smax_sb[:, cx : 2 * L : 2],
                        axis=mybir.AxisListType.X,
                    )
                    nc.scalar.activation(
                        res_sb[:, cx : cx + 1],
                        ssum[:],
                        AF.Copy,
                        bias=1.0,
                        scale=-1.0 / L,
                    )
                for cx in range(2):
                    nc.sync.dma_start(
                        out_ext.ap()[cx : cx + 1, :].rearrange("c p -> p c"),
                        res_sb[:, cx : cx + 1],
                    )

    nc.compile()
    return nc


_NC_CACHE = None


def _get_nc():
    global _NC_CACHE
    if _NC_CACHE is None:
        _patch_ldwopt()
        _NC_CACHE = build()
    return _NC_CACHE


def _prep_shard(g_lp, c):
    # [L, MS, D] slice -> [L, 128, KC, MS] (contiguous partition lines)
    sl = g_lp[:, c * MS : (c + 1) * MS, :]  # [L, MS, D]
    # d = k*128 + p  ->  [L, MS, KC, 128] -> [L, 128, KC, MS]
    return np.ascontiguousarray(
        sl.reshape(L, MS, KC, 128).transpose(0, 3, 2, 1)
    )


def _prep_inputs(test_patch_tokens, normal_patch_tokens):
    q = np.asarray(test_patch_tokens, dtype=np.float32)
    g = np.asarray(normal_patch_tokens, dtype=np.float32)
    qn_lp = q.astype(NP_IN)  # [L, N, D]
    # qn packed [128, 2L, D]: [p, l*2+c, d] = qn[l, c*128+p, d]
    qn_pk = np.ascontiguousarray(
        qn_lp.reshape(L, 2, 128, D).transpose(2, 0, 1, 3).reshape(128, 2 * L, D)
    )
    # qt packed [128, L*KC, N]: [p, l*KC+k, n] = qn[l, n, k*128+p]
    qt_pk = np.ascontiguousarray(
        qn_lp.transpose(0, 2, 1)  # [L, D, N]
        .reshape(L, KC, 128, N)
        .transpose(2, 0, 1, 3)
        .reshape(128, L * KC, N)
    )
    g_lp = g.astype(NP_IN)  # [L, M, D]
    with ThreadPoolExecutor(NCORES) as ex:
        shards = list(ex.map(lambda c: _prep_shard(g_lp, c), range(NCORES)))
    return [
        {"g_t": shards[c], "q_t": qt_pk, "q_n": qn_pk} for c in range(NCORES)
    ]


def kernel(test_patch_tokens: np.ndarray, normal_patch_tokens: np.ndarray):
    in_maps = _prep_inputs(test_patch_tokens, normal_patch_tokens)
    nc = _get_nc()
    results = run_bass_kernel_spmd(nc, in_maps, core_ids=list(range(NCORES))).results
    if HOST_COMBINE:
        # gather/unshard: global max over the 8 gallery shards, then the
        # tiny epilogue (1/||q|| scale, mean over layers, 1-x)
        lmax = np.max(
            np.stack([results[c]["out_lmax"] for c in range(NCORES)]), axis=0
        )  # [128, 2L]: column j = layer*2 + chunk
        invq = results[0]["out_invq"]  # identical on all cores
        smax = lmax * invq
        test_sim = smax.reshape(128, L, 2).mean(axis=1)  # [128(p), 2(chunk)]
        out = 1.0 - test_sim.T.reshape(N)  # n = chunk*128 + p
        return out.astype(np.float32).reshape(1, 1, 16, 16)
    out = results[0]["out"].astype(np.float32).reshape(1, 1, 16, 16)
    return out


# revision 16
# speedup vs baseline: 1.3907x; 1.2018x over previous
"""Distributed Trainium2 kernel for AnomalyMoE k-NN retrieval.

reference:  q = l2norm(test[L,N,D]); g = l2norm(normal[L,M,D])
            sim[l,n,m] = q . g ; out = (1 - mean_l max_m sim).reshape(1,1,16,16)

Strategy (8 NeuronCores):
- Shard gallery along M (6400 rows/core). Host pre-packs each shard to
  [L, 128, KC, MS] fp8e4m3 so each full-layer DMA moves 51.2KB fully
  contiguous per partition (near-peak HBM efficiency); a small starter
  chunk covers the first two supers so compute starts early.
- Per core: dot[n,m] accumulated on TensorE with fp8 DoubleRow pair-matmuls
  (contraction 256/instruction), supers processed in PAIRS so consecutive
  matmuls share a stationary operand (walrus --enable-ldw-opt dedups the
  redundant LDWEIGHTS).  Gallery row norms via Square + ones-DoubleRow-
  matmul, then ACT Abs_reciprocal_sqrt.  Squares split ACT/DVE
  (KERNEL_SQ_ACT); running per-layer max on DVE or the SDMA CCE datapath
  (KERNEL_MAX_ENG).
- Queries are NOT normalized on the way in: 1/||q_n|| is applied to the
  per-layer maxes at the end (positive per-query scale commutes with max).
- AllReduce(max) over 8 cores per layer (overlapped), or host-side combine
  of per-shard maxes (KERNEL_HOST_COMBINE).
"""

import os
import sys
from concurrent.futures import ThreadPoolExecutor

sys.path.insert(0, "/opt/trn_rl_repo")

import numpy as np
import ml_dtypes

import concourse.bacc as bacc
import concourse.mybir as mybir
import concourse.tile as tile
import concourse.bass_utils as bass_utils
from concourse.bass_utils import run_bass_kernel_spmd

F32 = mybir.dt.float32
BF16 = mybir.dt.bfloat16
AF = mybir.ActivationFunctionType
DR = mybir.MatmulPerfMode.DoubleRow

DT_IN = mybir.dt.float8e4
NP_IN = ml_dtypes.float8_e4m3fn

NCORES = 8
L = 4
D = 1024
N = 256
M_FULL = 51200
MS = M_FULL // NCORES  # 6400 per core
KC = D // 128  # 8 contraction chunks of 128
KP = KC // 2  # 4 DoubleRow pairs
SUPER = 512
STARTER = 1024  # first two supers of layer 0 come via a small early DMA

SKEW_PAIRS = int(os.environ.get("KERNEL_SKEW", "1"))  # pipeline depth in pairs
SQ_ACT = int(os.environ.get("KERNEL_SQ_ACT", "5"))  # k-chunks squared on ACT
SQ_ALT = int(os.environ.get("KERNEL_SQ_ALT", "0"))  # alternate SQ_ACT/SQ_ACT+1
MAX_ENG = os.environ.get("KERNEL_MAX_ENG", "dve")  # dve | dma
SPLIT_CC = os.environ.get("KERNEL_SPLIT_CC", "1") == "1"
HOST_COMBINE = os.environ.get("KERNEL_HOST_COMBINE", "0") == "1"
LDWOPT = os.environ.get("KERNEL_LDWOPT", "0") == "1"
BUFS_SQ = int(os.environ.get("KERNEL_BUFS_SQ", "4"))
BUFS_SIM = int(os.environ.get("KERNEL_BUFS_SIM", "4"))
KERNEL_TAG = os.environ.get("KERNEL_TAG", "")
NEG = -3.0e38

_LDW_PATCHED = False


def _patch_ldwopt():
    # walrus skips redundant LDWEIGHTS when consecutive matmuls share a
    # stationary operand; the flag is off in bass_utils' default cmdline.
    global _LDW_PATCHED
    if _LDW_PATCHED or not LDWOPT:
        return
    orig = bass_utils.run_command

    def patched(cmd, **kw):
        cmd = [
            "--enable-ldw-opt=true" if c == "--enable-ldw-opt=false" else c
            for c in cmd
        ]
        return orig(cmd, **kw)

    bass_utils.run_command = patched
    _LDW_PATCHED = True


def build():
    nc = bacc.Bacc("TRN2", target_bir_lowering=False, debug=False, num_devices=NCORES)
    g_ext = nc.dram_tensor("g_t", [L, 128, KC, MS], DT_IN, kind="ExternalInput")
    qt_ext = nc.dram_tensor("q_t", [128, L * KC, N], DT_IN, kind="ExternalInput")
    qn_ext = nc.dram_tensor("q_n", [128, 2 * L, D], DT_IN, kind="ExternalInput")
    if HOST_COMBINE:
        lmax_ext = nc.dram_tensor("out_lmax", [128, 2 * L], F32, kind="ExternalOutput")
        invq_ext = nc.dram_tensor("out_invq", [128, 2 * L], F32, kind="ExternalOutput")
        out_ext = cc_in = cc_out = None
    else:
        out_ext = nc.dram_tensor("out", [2, 128], F32, kind="ExternalOutput")
        cc_in = nc.dram_tensor("cc_in", [2 * L, 128], F32)
        cc_out = nc.dram_tensor("cc_out", [2 * L, 128], F32, addr_space="Shared")
        lmax_ext = invq_ext = None

    with tile.TileContext(nc) as tc:
        with (
            tc.tile_pool(name="persist", bufs=1) as pp,
            tc.tile_pool(name="glp", bufs=2) as glpool,
            tc.tile_pool(name="sqp", bufs=BUFS_SQ) as sqpool,
            tc.tile_pool(name="invgp", bufs=4) as invgpool,
            tc.tile_pool(name="simp", bufs=BUFS_SIM) as simpool,
            tc.tile_pool(name="qsqp", bufs=2) as qsqpool,
            tc.tile_pool(name="pm0", bufs=3, space="PSUM") as pm0pool,
            tc.tile_pool(name="pm1", bufs=3, space="PSUM") as pm1pool,
            tc.tile_pool(name="pnorm", bufs=2, space="PSUM") as pnormpool,
        ):
            # ---- persistent tiles ----
            qt_sb = pp.tile([128, L * KC, N], DT_IN, name="qt_sb")
            qn_sb = pp.tile([128, 2 * L, D], DT_IN, name="qn_sb")
            ones_sb = pp.tile([128, 2, 128], DT_IN, name="ones_sb")
            nc.gpsimd.memset(ones_sb[:], 1.0)
            runmax = pp.tile([128, 2 * L, SUPER], BF16, name="runmax")
            nc.gpsimd.memset(runmax[:], NEG)
            starter = pp.tile([128, KC, STARTER], DT_IN, name="starter")
            qss = pp.tile([128, 2 * L], F32, name="qss")
            invq = pp.tile([128, 2 * L], F32, name="invq")
            lmax_sb = pp.tile([128, 2 * L], F32, name="lmax_sb")
            gmax_sb = pp.tile([128, 2 * L], F32, name="gmax_sb")
            smax_sb = pp.tile([128, 2 * L], F32, name="smax_sb")
            res_sb = pp.tile([128, 2], F32, name="res_sb")
            if KERNEL_TAG:
                tag_sb = pp.tile([128, 1], F32, name=f"tag_{KERNEL_TAG}")
                nc.gpsimd.memset(tag_sb[:], 1.0)

            def emit_q_norm_step(step):
                # spread query-norm work through the main loop (ACT bubbles)
                if step == 0:
                    nc.sync.dma_start(qn_sb[:], qn_ext.ap())
                elif step <= 2 * L:
                    j = step - 1
                    qsq_scr = qsqpool.tile([128, D], BF16, name="qsq_scr")
                    nc.scalar.activation(
                        qsq_scr[:],
                        qn_sb[:, j, :],
                        AF.Square,
                        accum_out=qss[:, j : j + 1],
                    )
                elif step == 2 * L + 1:
                    nc.scalar.activation(invq[:], qss[:], AF.Abs_reciprocal_sqrt)
                    if HOST_COMBINE:
                        nc.sync.dma_start(invq_ext.ap(), invq[:])

            def stage_a(glt, m0, msz, sidx=0):
                # squares: sq[:, k, :msz] = g^2, split ACT / DVE
                sq = sqpool.tile([128, KC, SUPER], DT_IN, name="sq")
                a = SQ_ACT + (1 if SQ_ALT and sidx % 2 else 0)
                if a:
                    nc.scalar.activation(
                        sq[:, :a, :msz], glt[:, :a, m0 : m0 + msz], AF.Square
                    )
                if a < KC:
                    nc.vector.tensor_tensor(
                        out=sq[:, a:, :msz],
                        in0=glt[:, a:, m0 : m0 + msz],
                        in1=glt[:, a:, m0 : m0 + msz],
                        op=mybir.AluOpType.mult,
                    )
                return sq

            def stage_b_pair(items):
                # items: list of (lx, glt, m0, msz, sq), same layer, len 1-2.
                # grouped so consecutive matmuls share stationaries.
                pns = []
                for lx, glt, m0, msz, sq in items:
                    pnorm = pnormpool.tile([128, SUPER], F32, name="pnorm")
                    pns.append(pnorm)
                for j in range(KP):
                    for (lx, glt, m0, msz, sq), pnorm in zip(items, pns):
                        nc.tensor.matmul(
                            pnorm[:, :msz],
                            ones_sb[:],
                            sq[:, 2 * j : 2 * j + 2, :msz],
                            start=(j == 0),
                            stop=(j == KP - 1),
                            perf_mode=DR,
                        )
                invgs = []
                for (lx, glt, m0, msz, sq), pnorm in zip(items, pns):
                    invg = invgpool.tile([128, SUPER], F32, name="invg")
                    nc.scalar.activation(
                        invg[:, :msz], pnorm[:, :msz], AF.Abs_reciprocal_sqrt
                    )
                    invgs.append(invg)

                lx = items[0][0]
                pms = {}
                for cx, pmpool in ((0, pm0pool), (1, pm1pool)):
                    for i, (lx_, glt, m0, msz, sq) in enumerate(items):
                        pms[(i, cx)] = pmpool.tile([128, SUPER], F32, name=f"pm{cx}")
                    for j in range(KP):
                        for i, (lx_, glt, m0, msz, sq) in enumerate(items):
                            nc.tensor.matmul(
                                pms[(i, cx)][:, :msz],
                                qt_sb[
                                    :,
                                    lx * KC + 2 * j : lx * KC + 2 * j + 2,
                                    cx * 128 : (cx + 1) * 128,
                                ],
                                glt[:, 2 * j : 2 * j + 2, m0 : m0 + msz],
                                start=(j == 0),
                                stop=(j == KP - 1),
                                perf_mode=DR,
                            )
                for i, (lx_, glt, m0, msz, sq) in enumerate(items):
                    for cx in range(2):
                        pm = pms[(i, cx)]
                        sim = simpool.tile([128, SUPER], BF16, name="sim")
                        nc.vector.tensor_tensor(
                            out=sim[:, :msz],
                            in0=pm[:, :msz],
                            in1=invgs[i][:, :msz],
                            op=mybir.AluOpType.mult,
                        )
                        j2 = lx * 2 + cx
                        if MAX_ENG == "dma":
                            nc.gpsimd.dma_start(
                                out=runmax[:, j2, :msz],
                                in_=sim[:, :msz],
                                accum_op=mybir.AluOpType.max,
                            )
                        else:
                            nc.vector.tensor_tensor(
                                out=runmax[:, j2, :msz],
                                in0=runmax[:, j2, :msz],
                                in1=sim[:, :msz],
                                op=mybir.AluOpType.max,
                            )

            def layer_done(lx):
                for cx in range(2):
                    j = lx * 2 + cx
                    nc.vector.reduce_max(
                        lmax_sb[:, j : j + 1],
                        runmax[:, j, :],
                        axis=mybir.AxisListType.X,
                    )
                if HOST_COMBINE:
                    nc.sync.dma_start(
                        lmax_ext.ap()[:, 2 * lx : 2 * lx + 2],
                        lmax_sb[:, 2 * lx : 2 * lx + 2],
                    )
                elif SPLIT_CC:
                    nc.sync.dma_start(
                        cc_in.ap()[2 * lx : 2 * lx + 2, :].rearrange("c p -> p c"),
                        lmax_sb[:, 2 * lx : 2 * lx + 2],
                    )
                    nc.gpsimd.collective_compute(
                        "AllReduce",
                        mybir.AluOpType.max,
                        replica_groups=[list(range(NCORES))],
                        ins=[cc_in.ap()[2 * lx : 2 * lx + 2, :].opt()],
                        outs=[cc_out.ap()[2 * lx : 2 * lx + 2, :].opt()],
                    )

            # ---- DMAs: starter chunk, then full contiguous layers ----
            big = {}

            def emit_big_dma(lx):
                m_lo = STARTER if lx == 0 else 0
                w = MS - m_lo
                glt = glpool.tile([128, KC, MS], DT_IN, name="gl")
                if lx == 0:
                    # split so early supers unblock before the full layer lands
                    h = w // 2 // SUPER * SUPER
                    nc.sync.dma_start(
                        glt[:, :, :h], g_ext.ap()[lx][:, :, m_lo : m_lo + h]
                    )
                    nc.sync.dma_start(
                        glt[:, :, h:w], g_ext.ap()[lx][:, :, m_lo + h : MS]
                    )
                else:
                    nc.sync.dma_start(
                        glt[:, :, :w], g_ext.ap()[lx][:, :, m_lo:MS]
                    )
                big[lx] = glt

            nc.sync.dma_start(starter[:], g_ext.ap()[0][:, :, :STARTER])
            nc.sync.dma_start(qt_sb[:], qt_ext.ap())
            emit_q_norm_step(0)  # qn DMA
            emit_big_dma(0)
            emit_big_dma(1)

            # work items: (lx, tile_kind, m0_in_tile, msz)
            def layer_supers(lx):
                out = []
                if lx == 0:
                    for m0 in range(0, STARTER, SUPER):
                        out.append((lx, "S", m0, SUPER))
                    base = STARTER
                else:
                    base = 0
                w = MS - base
                for m0 in range(0, w, SUPER):
                    out.append((lx, "B", m0, min(SUPER, w - m0)))
                return out

            pending = []
            done_lx = 0
            qn_step = 1
            sidx = 0
            for lx in range(L):
                for it in layer_supers(lx):
                    lx_, kind, m0, msz = it
                    if kind == "S":
                        glt = starter
                    else:
                        glt = big[lx_]
                    pending.append((lx_, glt, m0, msz, stage_a(glt, m0, msz, sidx)))
                    if sidx >= 2 and sidx % 2 == 0 and qn_step <= 2 * L + 1:
                        emit_q_norm_step(qn_step)
                        qn_step += 1
                    # pop a pair when we have SKEW_PAIRS*2 + 2 items pending
                    while len(pending) > 2 * SKEW_PAIRS + 1:
                        take = 2 if (
                            len(pending) >= 2 and pending[0][0] == pending[1][0]
                        ) else 1
                        batch, pending = pending[:take], pending[take:]
                        stage_b_pair(batch)
                        nxt = pending[0][0] if pending else L
                        while done_lx < nxt:
                            layer_done(done_lx)
                            done_lx += 1
                    sidx += 1
                if lx + 2 < L:
                    emit_big_dma(lx + 2)
            while pending:
                take = 2 if (
                    len(pending) >= 2 and pending[0][0] == pending[1][0]
                ) else 1
                batch, pending = pending[:take], pending[take:]
                stage_b_pair(batch)
                nxt = pending[0][0] if pending else L
                while done_lx < nxt:
                    layer_done(done_lx)
                    done_lx += 1
            while qn_step <= 2 * L + 1:
                emit_q_norm_step(qn_step)
                qn_step += 1

            if not HOST_COMBINE:
                if not SPLIT_CC:
                    nc.sync.dma_start(
                        cc_in.ap().rearrange("c p -> p c"), lmax_sb[:]
                    )
                    nc.gpsimd.collective_compute(
                        "AllReduce",
                        mybir.AluOpType.max,
                        replica_groups=[list(range(NCORES))],
                        ins=[cc_in.ap().opt()],
                        outs=[cc_out.ap().opt()],
                    )
                nc.sync.dma_start(
                    gmax_sb[:], cc_out.ap().rearrange("c p -> p c")
                )

                # ---- scale by 1/||q||, mean over layers, 1 - x ----
                nc.vector.tensor_tensor(
                    out=smax_sb[:],
                    in0=gmax_sb[:],
                    in1=invq[:],
                    op=mybir.AluOpType.mult,
                )
                for cx in range(2):
                    ssum = pp.tile([128, 1], F32, name=f"ssum{cx}")
                    nc.vector.reduce_sum(
                        ssum[:],
                        smax_sb[:, cx : 2 * L : 2],
                        axis=mybir.AxisListType.X,
                    )
                    nc.scalar.activation(
                        res_sb[:, cx : cx + 1],
                        ssum[:],
                        AF.Copy,
                        bias=1.0,
                        scale=-1.0 / L,
                    )
                for cx in range(2):
                    nc.sync.dma_start(
                        out_ext.ap()[cx : cx + 1, :].rearrange("c p -> p c"),
                        res_sb[:, cx : cx + 1],
                    )

    nc.compile()
    return nc


_NC_CACHE = None


def _get_nc():
    global _NC_CACHE
    if _NC_CACHE is None:
        _patch_ldwopt()
        _NC_CACHE = build()
    return _NC_CACHE


def _prep_shard(g_lp, c):
    # [L, MS, D] slice -> [L, 128, KC, MS] (contiguous partition lines)
    sl = g_lp[:, c * MS : (c + 1) * MS, :]  # [L, MS, D]
    # d = k*128 + p  ->  [L, MS, KC, 128] -> [L, 128, KC, MS]
    return np.ascontiguousarray(
        sl.reshape(L, MS, KC, 128).transpose(0, 3, 2, 1)
    )


def _prep_inputs(test_patch_tokens, normal_patch_tokens):
    q = np.asarray(test_patch_tokens, dtype=np.float32)
    g = np.asarray(normal_patch_tokens, dtype=np.float32)
    qn_lp = q.astype(NP_IN)  # [L, N, D]
    # qn packed [128, 2L, D]: [p, l*2+c, d] = qn[l, c*128+p, d]
    qn_pk = np.ascontiguousarray(
        qn_lp.reshape(L, 2, 128, D).transpose(2, 0, 1, 3).reshape(128, 2 * L, D)
    )
    # qt packed [128, L*KC, N]: [p, l*KC+k, n] = qn[l, n, k*128+p]
    qt_pk = np.ascontiguousarray(
        qn_lp.transpose(0, 2, 1)  # [L, D, N]
        .reshape(L, KC, 128, N)
        .transpose(2, 0, 1, 3)
        .reshape(128, L * KC, N)
    )
    g_lp = g.astype(NP_IN)  # [L, M, D]
    with ThreadPoolExecutor(NCORES) as ex:
        shards = list(ex.map(lambda c: _prep_shard(g_lp, c), range(NCORES)))
    return [
        {"g_t": shards[c], "q_t": qt_pk, "q_n": qn_pk} for c in range(NCORES)
    ]


def kernel(test_patch_tokens: np.ndarray, normal_patch_tokens: np.ndarray):
    in_maps = _prep_inputs(test_patch_tokens, normal_patch_tokens)
    nc = _get_nc()
    results = run_bass_kernel_spmd(nc, in_maps, core_ids=list(range(NCORES))).results
    if HOST_COMBINE:
        # gather/unshard: global max over the 8 gallery shards, then the
        # tiny epilogue (1/||q|| scale, mean over layers, 1-x)
        lmax = np.max(
            np.stack([results[c]["out_lmax"] for c in range(NCORES)]), axis=0
        )  # [128, 2L]: column j = layer*2 + chunk
        invq = results[0]["out_invq"]  # identical on all cores
        smax = lmax * invq
        test_sim = smax.reshape(128, L, 2).mean(axis=1)  # [128(p), 2(chunk)]
        out = 1.0 - test_sim.T.reshape(N)  # n = chunk*128 + p
        return out.astype(np.float32).reshape(1, 1, 16, 16)
    out = results[0]["out"].astype(np.float32).reshape(1, 1, 16, 16)
    return out


# revision 17
# speedup vs baseline: 1.4554x; 1.0466x over previous
"""Distributed Trainium2 kernel for AnomalyMoE k-NN retrieval.

reference:  q = l2norm(test[L,N,D]); g = l2norm(normal[L,M,D])
            sim[l,n,m] = q . g ; out = (1 - mean_l max_m sim).reshape(1,1,16,16)

Strategy (8 NeuronCores):
- Shard gallery along M (6400 rows/core). Host pre-packs each shard to
  [L, 128, KC, MS] fp8e4m3 so each full-layer DMA moves 51.2KB fully
  contiguous per partition (near-peak HBM efficiency); a small starter
  chunk covers the first two supers so compute starts early.
- Per core: dot[n,m] accumulated on TensorE with fp8 DoubleRow pair-matmuls
  (contraction 256/instruction), supers processed in PAIRS so consecutive
  matmuls share a stationary operand (walrus --enable-ldw-opt dedups the
  redundant LDWEIGHTS).  Gallery row norms via Square + ones-DoubleRow-
  matmul, then ACT Abs_reciprocal_sqrt.  Squares split ACT/DVE
  (KERNEL_SQ_ACT); running per-layer max on DVE or the SDMA CCE datapath
  (KERNEL_MAX_ENG).
- Queries are NOT normalized on the way in: 1/||q_n|| is applied to the
  per-layer maxes at the end (positive per-query scale commutes with max).
- AllReduce(max) over 8 cores per layer (overlapped), or host-side combine
  of per-shard maxes (KERNEL_HOST_COMBINE).
"""

import os
import sys
from concurrent.futures import ThreadPoolExecutor

sys.path.insert(0, "/opt/trn_rl_repo")

import numpy as np
import ml_dtypes

import concourse.bacc as bacc
import concourse.mybir as mybir
import concourse.tile as tile
import concourse.bass_utils as bass_utils
from concourse.bass_utils import run_bass_kernel_spmd

F32 = mybir.dt.float32
BF16 = mybir.dt.bfloat16
AF = mybir.ActivationFunctionType
DR = mybir.MatmulPerfMode.DoubleRow

DT_IN = mybir.dt.float8e4
NP_IN = ml_dtypes.float8_e4m3fn

NCORES = 8
L = 4
D = 1024
N = 256
M_FULL = 51200
MS = M_FULL // NCORES  # 6400 per core
KC = D // 128  # 8 contraction chunks of 128
KP = KC // 2  # 4 DoubleRow pairs
SUPER = 512
STARTER = 1024  # first two supers of layer 0 come via a small early DMA

SKEW_PAIRS = int(os.environ.get("KERNEL_SKEW", "1"))  # pipeline depth in pairs
SQ_ACT = int(os.environ.get("KERNEL_SQ_ACT", "5"))  # k-chunks squared on ACT
SQ_ALT = int(os.environ.get("KERNEL_SQ_ALT", "0"))  # alternate SQ_ACT/SQ_ACT+1
MAX_ENG = os.environ.get("KERNEL_MAX_ENG", "dve")  # dve | dma
SPLIT_CC = os.environ.get("KERNEL_SPLIT_CC", "1") == "1"
HOST_COMBINE = os.environ.get("KERNEL_HOST_COMBINE", "1") == "1"
LDWOPT = os.environ.get("KERNEL_LDWOPT", "0") == "1"
BUFS_SQ = int(os.environ.get("KERNEL_BUFS_SQ", "4"))
BUFS_SIM = int(os.environ.get("KERNEL_BUFS_SIM", "4"))
KERNEL_TAG = os.environ.get("KERNEL_TAG", "")
NEG = -3.0e38

_LDW_PATCHED = False


def _patch_ldwopt():
    # walrus skips redundant LDWEIGHTS when consecutive matmuls share a
    # stationary operand; the flag is off in bass_utils' default cmdline.
    global _LDW_PATCHED
    if _LDW_PATCHED or not LDWOPT:
        return
    orig = bass_utils.run_command

    def patched(cmd, **kw):
        cmd = [
            "--enable-ldw-opt=true" if c == "--enable-ldw-opt=false" else c
            for c in cmd
        ]
        return orig(cmd, **kw)

    bass_utils.run_command = patched
    _LDW_PATCHED = True


def build():
    nc = bacc.Bacc("TRN2", target_bir_lowering=False, debug=False, num_devices=NCORES)
    g_ext = nc.dram_tensor("g_t", [L, 128, KC, MS], DT_IN, kind="ExternalInput")
    qt_ext = nc.dram_tensor("q_t", [128, L * KC, N], DT_IN, kind="ExternalInput")
    qn_ext = nc.dram_tensor("q_n", [128, 2 * L, D], DT_IN, kind="ExternalInput")
    if HOST_COMBINE:
        lmax_ext = nc.dram_tensor("out_lmax", [128, 2 * L], F32, kind="ExternalOutput")
        invq_ext = nc.dram_tensor("out_invq", [128, 2 * L], F32, kind="ExternalOutput")
        out_ext = cc_in = cc_out = None
    else:
        out_ext = nc.dram_tensor("out", [2, 128], F32, kind="ExternalOutput")
        cc_in = nc.dram_tensor("cc_in", [2 * L, 128], F32)
        cc_out = nc.dram_tensor("cc_out", [2 * L, 128], F32, addr_space="Shared")
        lmax_ext = invq_ext = None

    with tile.TileContext(nc) as tc:
        with (
            tc.tile_pool(name="persist", bufs=1) as pp,
            tc.tile_pool(name="glp", bufs=2) as glpool,
            tc.tile_pool(name="sqp", bufs=BUFS_SQ) as sqpool,
            tc.tile_pool(name="invgp", bufs=4) as invgpool,
            tc.tile_pool(name="simp", bufs=BUFS_SIM) as simpool,
            tc.tile_pool(name="qsqp", bufs=2) as qsqpool,
            tc.tile_pool(name="pm0", bufs=3, space="PSUM") as pm0pool,
            tc.tile_pool(name="pm1", bufs=3, space="PSUM") as pm1pool,
            tc.tile_pool(name="pnorm", bufs=2, space="PSUM") as pnormpool,
        ):
            # ---- persistent tiles ----
            qt_sb = pp.tile([128, L * KC, N], DT_IN, name="qt_sb")
            qn_sb = pp.tile([128, 2 * L, D], DT_IN, name="qn_sb")
            ones_sb = pp.tile([128, 2, 128], DT_IN, name="ones_sb")
            nc.gpsimd.memset(ones_sb[:], 1.0)
            runmax = pp.tile([128, 2 * L, SUPER], BF16, name="runmax")
            nc.gpsimd.memset(runmax[:], NEG)
            starter = pp.tile([128, KC, STARTER], DT_IN, name="starter")
            qss = pp.tile([128, 2 * L], F32, name="qss")
            invq = pp.tile([128, 2 * L], F32, name="invq")
            lmax_sb = pp.tile([128, 2 * L], F32, name="lmax_sb")
            gmax_sb = pp.tile([128, 2 * L], F32, name="gmax_sb")
            smax_sb = pp.tile([128, 2 * L], F32, name="smax_sb")
            res_sb = pp.tile([128, 2], F32, name="res_sb")
            if KERNEL_TAG:
                tag_sb = pp.tile([128, 1], F32, name=f"tag_{KERNEL_TAG}")
                nc.gpsimd.memset(tag_sb[:], 1.0)

            def emit_q_norm_step(step):
                # spread query-norm work through the main loop (ACT bubbles)
                if step == 0:
                    nc.sync.dma_start(qn_sb[:], qn_ext.ap())
                elif step <= 2 * L:
                    j = step - 1
                    qsq_scr = qsqpool.tile([128, D], BF16, name="qsq_scr")
                    nc.scalar.activation(
                        qsq_scr[:],
                        qn_sb[:, j, :],
                        AF.Square,
                        accum_out=qss[:, j : j + 1],
                    )
                elif step == 2 * L + 1:
                    nc.scalar.activation(invq[:], qss[:], AF.Abs_reciprocal_sqrt)
                    if HOST_COMBINE:
                        nc.sync.dma_start(invq_ext.ap(), invq[:])

            def stage_a(glt, m0, msz, sidx=0):
                # squares: sq[:, k, :msz] = g^2, split ACT / DVE
                sq = sqpool.tile([128, KC, SUPER], DT_IN, name="sq")
                a = SQ_ACT + (1 if SQ_ALT and sidx % 2 else 0)
                if a:
                    nc.scalar.activation(
                        sq[:, :a, :msz], glt[:, :a, m0 : m0 + msz], AF.Square
                    )
                if a < KC:
                    nc.vector.tensor_tensor(
                        out=sq[:, a:, :msz],
                        in0=glt[:, a:, m0 : m0 + msz],
                        in1=glt[:, a:, m0 : m0 + msz],
                        op=mybir.AluOpType.mult,
                    )
                return sq

            def stage_b_pair(items):
                # items: list of (lx, glt, m0, msz, sq), same layer, len 1-2.
                # grouped so consecutive matmuls share stationaries.
                pns = []
                for lx, glt, m0, msz, sq in items:
                    pnorm = pnormpool.tile([128, SUPER], F32, name="pnorm")
                    pns.append(pnorm)
                for j in range(KP):
                    for (lx, glt, m0, msz, sq), pnorm in zip(items, pns):
                        nc.tensor.matmul(
                            pnorm[:, :msz],
                            ones_sb[:],
                            sq[:, 2 * j : 2 * j + 2, :msz],
                            start=(j == 0),
                            stop=(j == KP - 1),
                            perf_mode=DR,
                        )
                invgs = []
                for (lx, glt, m0, msz, sq), pnorm in zip(items, pns):
                    invg = invgpool.tile([128, SUPER], F32, name="invg")
                    nc.scalar.activation(
                        invg[:, :msz], pnorm[:, :msz], AF.Abs_reciprocal_sqrt
                    )
                    invgs.append(invg)

                lx = items[0][0]
                pms = {}
                for cx, pmpool in ((0, pm0pool), (1, pm1pool)):
                    for i, (lx_, glt, m0, msz, sq) in enumerate(items):
                        pms[(i, cx)] = pmpool.tile([128, SUPER], F32, name=f"pm{cx}")
                    for j in range(KP):
                        for i, (lx_, glt, m0, msz, sq) in enumerate(items):
                            nc.tensor.matmul(
                                pms[(i, cx)][:, :msz],
                                qt_sb[
                                    :,
                                    lx * KC + 2 * j : lx * KC + 2 * j + 2,
                                    cx * 128 : (cx + 1) * 128,
                                ],
                                glt[:, 2 * j : 2 * j + 2, m0 : m0 + msz],
                                start=(j == 0),
                                stop=(j == KP - 1),
                                perf_mode=DR,
                            )
                for i, (lx_, glt, m0, msz, sq) in enumerate(items):
                    for cx in range(2):
                        pm = pms[(i, cx)]
                        sim = simpool.tile([128, SUPER], BF16, name="sim")
                        nc.vector.tensor_tensor(
                            out=sim[:, :msz],
                            in0=pm[:, :msz],
                            in1=invgs[i][:, :msz],
                            op=mybir.AluOpType.mult,
                        )
                        j2 = lx * 2 + cx
                        if MAX_ENG == "dma":
                            nc.gpsimd.dma_start(
                                out=runmax[:, j2, :msz],
                                in_=sim[:, :msz],
                                accum_op=mybir.AluOpType.max,
                            )
                        else:
                            nc.vector.tensor_tensor(
                                out=runmax[:, j2, :msz],
                                in0=runmax[:, j2, :msz],
                                in1=sim[:, :msz],
                                op=mybir.AluOpType.max,
                            )

            def layer_done(lx):
                for cx in range(2):
                    j = lx * 2 + cx
                    nc.vector.reduce_max(
                        lmax_sb[:, j : j + 1],
                        runmax[:, j, :],
                        axis=mybir.AxisListType.X,
                    )
                if HOST_COMBINE:
                    nc.sync.dma_start(
                        lmax_ext.ap()[:, 2 * lx : 2 * lx + 2],
                        lmax_sb[:, 2 * lx : 2 * lx + 2],
                    )
                elif SPLIT_CC:
                    nc.sync.dma_start(
                        cc_in.ap()[2 * lx : 2 * lx + 2, :].rearrange("c p -> p c"),
                        lmax_sb[:, 2 * lx : 2 * lx + 2],
                    )
                    nc.gpsimd.collective_compute(
                        "AllReduce",
                        mybir.AluOpType.max,
                        replica_groups=[list(range(NCORES))],
                        ins=[cc_in.ap()[2 * lx : 2 * lx + 2, :].opt()],
                        outs=[cc_out.ap()[2 * lx : 2 * lx + 2, :].opt()],
                    )

            # ---- DMAs: starter chunk, then full contiguous layers ----
            big = {}

            def emit_big_dma(lx):
                m_lo = STARTER if lx == 0 else 0
                w = MS - m_lo
                glt = glpool.tile([128, KC, MS], DT_IN, name="gl")
                if lx == 0:
                    # split so early supers unblock before the full layer lands
                    h = w // 2 // SUPER * SUPER
                    nc.sync.dma_start(
                        glt[:, :, :h], g_ext.ap()[lx][:, :, m_lo : m_lo + h]
                    )
                    nc.sync.dma_start(
                        glt[:, :, h:w], g_ext.ap()[lx][:, :, m_lo + h : MS]
                    )
                else:
                    nc.sync.dma_start(
                        glt[:, :, :w], g_ext.ap()[lx][:, :, m_lo:MS]
                    )
                big[lx] = glt

            nc.sync.dma_start(starter[:], g_ext.ap()[0][:, :, :STARTER])
            nc.sync.dma_start(qt_sb[:], qt_ext.ap())
            emit_q_norm_step(0)  # qn DMA
            emit_big_dma(0)
            emit_big_dma(1)

            # work items: (lx, tile_kind, m0_in_tile, msz)
            def layer_supers(lx):
                out = []
                if lx == 0:
                    for m0 in range(0, STARTER, SUPER):
                        out.append((lx, "S", m0, SUPER))
                    base = STARTER
                else:
                    base = 0
                w = MS - base
                for m0 in range(0, w, SUPER):
                    out.append((lx, "B", m0, min(SUPER, w - m0)))
                return out

            pending = []
            done_lx = 0
            qn_step = 1
            sidx = 0
            for lx in range(L):
                for it in layer_supers(lx):
                    lx_, kind, m0, msz = it
                    if kind == "S":
                        glt = starter
                    else:
                        glt = big[lx_]
                    pending.append((lx_, glt, m0, msz, stage_a(glt, m0, msz, sidx)))
                    if sidx >= 2 and sidx % 2 == 0 and qn_step <= 2 * L + 1:
                        emit_q_norm_step(qn_step)
                        qn_step += 1
                    # pop a pair when we have SKEW_PAIRS*2 + 2 items pending
                    while len(pending) > 2 * SKEW_PAIRS + 1:
                        take = 2 if (
                            len(pending) >= 2 and pending[0][0] == pending[1][0]
                        ) else 1
                        batch, pending = pending[:take], pending[take:]
                        stage_b_pair(batch)
                        nxt = pending[0][0] if pending else L
                        while done_lx < nxt:
                            layer_done(done_lx)
                            done_lx += 1
                    sidx += 1
                if lx + 2 < L:
                    emit_big_dma(lx + 2)
            while pending:
                take = 2 if (
                    len(pending) >= 2 and pending[0][0] == pending[1][0]
                ) else 1
                batch, pending = pending[:take], pending[take:]
                stage_b_pair(batch)
                nxt = pending[0][0] if pending else L
                while done_lx < nxt:
                    layer_done(done_lx)
                    done_lx += 1
            while qn_step <= 2 * L + 1:
                emit_q_norm_step(qn_step)
                qn_step += 1

            if not HOST_COMBINE:
                if not SPLIT_CC:
                    nc.sync.dma_start(
                        cc_in.ap().rearrange("c p -> p c"), lmax_sb[:]
                    )
                    nc.gpsimd.collective_compute(
                        "AllReduce",
                        mybir.AluOpType.max,
                        replica_groups=[list(range(NCORES))],
                        ins=[cc_in.ap().opt()],
                        outs=[cc_out.ap().opt()],
                    )
                nc.sync.dma_start(
                    gmax_sb[:], cc_out.ap().rearrange("c p -> p c")
                )

                # ---- scale by 1/||q||, mean over layers, 1 - x ----
                nc.vector.tensor_tensor(
                    out=smax_sb[:],
                    in0=gmax_sb[:],
                    in1=invq[:],
                    op=mybir.AluOpType.mult,
                )
                for cx in range(2):
                    ssum = pp.tile([128, 1], F32, name=f"ssum{cx}")
                    nc.vector.reduce_sum(
                        ssum[:],
                        smax_sb[:, cx : 2 * L : 2],
                        axis=mybir.AxisListType.X,
                    )
                    nc.scalar.activation(
                        res_sb[:, cx : cx + 1],
                        ssum[:],
                        AF.Copy,
                        bias=1.0,
                        scale=-1.0 / L,
                    )
                for cx in range(2):
                    nc.sync.dma_start(
                        out_ext.ap()[cx : cx + 1, :].rearrange("c p -> p c"),
                        res_sb[:, cx : cx + 1],
                    )

    nc.compile()
    return nc


_NC_CACHE = None


def _get_nc():
    global _NC_CACHE
    if _NC_CACHE is None:
        _patch_ldwopt()
        _NC_CACHE = build()
    return _NC_CACHE


def _prep_shard(g_lp, c):
    # [L, MS, D] slice -> [L, 128, KC, MS] (contiguous partition lines)
    sl = g_lp[:, c * MS : (c + 1) * MS, :]  # [L, MS, D]
    # d = k*128 + p  ->  [L, MS, KC, 128] -> [L, 128, KC, MS]
    return np.ascontiguousarray(
        sl.reshape(L, MS, KC, 128).transpose(0, 3, 2, 1)
    )


def _prep_inputs(test_patch_tokens, normal_patch_tokens):
    q = np.asarray(test_patch_tokens, dtype=np.float32)
    g = np.asarray(normal_patch_tokens, dtype=np.float32)
    qn_lp = q.astype(NP_IN)  # [L, N, D]
    # qn packed [128, 2L, D]: [p, l*2+c, d] = qn[l, c*128+p, d]
    qn_pk = np.ascontiguousarray(
        qn_lp.reshape(L, 2, 128, D).transpose(2, 0, 1, 3).reshape(128, 2 * L, D)
    )
    # qt packed [128, L*KC, N]: [p, l*KC+k, n] = qn[l, n, k*128+p]
    qt_pk = np.ascontiguousarray(
        qn_lp.transpose(0, 2, 1)  # [L, D, N]
        .reshape(L, KC, 128, N)
        .transpose(2, 0, 1, 3)
        .reshape(128, L * KC, N)
    )
    g_lp = g.astype(NP_IN)  # [L, M, D]
    with ThreadPoolExecutor(NCORES) as ex:
        shards = list(ex.map(lambda c: _prep_shard(g_lp, c), range(NCORES)))
    return [
        {"g_t": shards[c], "q_t": qt_pk, "q_n": qn_pk} for c in range(NCORES)
    ]


def kernel(test_patch_tokens: np.ndarray, normal_patch_tokens: np.ndarray):
    in_maps = _prep_inputs(test_patch_tokens, normal_patch_tokens)
    nc = _get_nc()
    results = run_bass_kernel_spmd(nc, in_maps, core_ids=list(range(NCORES))).results
    if HOST_COMBINE:
        # gather/unshard: global max over the 8 gallery shards, then the
        # tiny epilogue (1/||q|| scale, mean over layers, 1-x)
        lmax = np.max(
            np.stack([results[c]["out_lmax"] for c in range(NCORES)]), axis=0
        )  # [128, 2L]: column j = layer*2 + chunk
        invq = results[0]["out_invq"]  # identical on all cores
        smax = lmax * invq
        test_sim = smax.reshape(128, L, 2).mean(axis=1)  # [128(p), 2(chunk)]
        out = 1.0 - test_sim.T.reshape(N)  # n = chunk*128 + p
        return out.astype(np.float32).reshape(1, 1, 16, 16)
    out = results[0]["out"].astype(np.float32).reshape(1, 1, 16, 16)
    return out


# revision 19
# speedup vs baseline: 1.4977x; 1.0290x over previous
"""Distributed Trainium2 kernel for AnomalyMoE k-NN retrieval.

reference:  q = l2norm(test[L,N,D]); g = l2norm(normal[L,M,D])
            sim[l,n,m] = q . g ; out = (1 - mean_l max_m sim).reshape(1,1,16,16)

Strategy (8 NeuronCores):
- Shard gallery along M (6400 rows/core). Host pre-packs each shard to
  [L, 128, KC, MS] fp8e4m3 so each full-layer DMA moves 51.2KB fully
  contiguous per partition (near-peak HBM efficiency); a small starter
  chunk covers the first two supers so compute starts early.
- Per core: dot[n,m] accumulated on TensorE with fp8 DoubleRow pair-matmuls
  (contraction 256/instruction), supers processed in PAIRS so consecutive
  matmuls share a stationary operand (walrus --enable-ldw-opt dedups the
  redundant LDWEIGHTS).  Gallery row norms via Square + ones-DoubleRow-
  matmul, then ACT Abs_reciprocal_sqrt.  Squares split ACT/DVE
  (KERNEL_SQ_ACT); running per-layer max on DVE or the SDMA CCE datapath
  (KERNEL_MAX_ENG).
- Queries are NOT normalized on the way in: 1/||q_n|| is applied to the
  per-layer maxes at the end (positive per-query scale commutes with max).
- AllReduce(max) over 8 cores per layer (overlapped), or host-side combine
  of per-shard maxes (KERNEL_HOST_COMBINE).
"""

import os
import sys
from concurrent.futures import ThreadPoolExecutor

sys.path.insert(0, "/opt/trn_rl_repo")

import numpy as np
import ml_dtypes

import concourse.bacc as bacc
import concourse.mybir as mybir
import concourse.tile as tile
import concourse.bass_utils as bass_utils
from concourse.bass_utils import run_bass_kernel_spmd

F32 = mybir.dt.float32
BF16 = mybir.dt.bfloat16
AF = mybir.ActivationFunctionType
DR = mybir.MatmulPerfMode.DoubleRow

DT_IN = mybir.dt.float8e4
NP_IN = ml_dtypes.float8_e4m3fn

NCORES = 8
L = 4
D = 1024
N = 256
M_FULL = 51200
MS = M_FULL // NCORES  # 6400 per core
KC = D // 128  # 8 contraction chunks of 128
KP = KC // 2  # 4 DoubleRow pairs
SUPER = 512
STARTER = 1024  # first two supers of layer 0 come via a small early DMA

SKEW_PAIRS = int(os.environ.get("KERNEL_SKEW", "1"))  # pipeline depth in pairs
SQ_ACT = int(os.environ.get("KERNEL_SQ_ACT", "5"))  # k-chunks squared on ACT
SQ_ALT = int(os.environ.get("KERNEL_SQ_ALT", "0"))  # alternate SQ_ACT/SQ_ACT+1
MAX_ENG = os.environ.get("KERNEL_MAX_ENG", "dve")  # dve | dma
SPLIT_CC = os.environ.get("KERNEL_SPLIT_CC", "1") == "1"
HOST_COMBINE = os.environ.get("KERNEL_HOST_COMBINE", "1") == "1"
LDWOPT = os.environ.get("KERNEL_LDWOPT", "0") == "1"
BUFS_SQ = int(os.environ.get("KERNEL_BUFS_SQ", "4"))
BUFS_SIM = int(os.environ.get("KERNEL_BUFS_SIM", "4"))
KERNEL_TAG = os.environ.get("KERNEL_TAG", "")
NEG = -3.0e38

_LDW_PATCHED = False


def _patch_ldwopt():
    # walrus skips redundant LDWEIGHTS when consecutive matmuls share a
    # stationary operand; the flag is off in bass_utils' default cmdline.
    global _LDW_PATCHED
    if _LDW_PATCHED or not LDWOPT:
        return
    orig = bass_utils.run_command

    def patched(cmd, **kw):
        cmd = [
            "--enable-ldw-opt=true" if c == "--enable-ldw-opt=false" else c
            for c in cmd
        ]
        return orig(cmd, **kw)

    bass_utils.run_command = patched
    _LDW_PATCHED = True


def build():
    nc = bacc.Bacc("TRN2", target_bir_lowering=False, debug=False, num_devices=NCORES)
    g_ext = nc.dram_tensor("g_t", [L, 128, KC, MS], DT_IN, kind="ExternalInput")
    qt_ext = nc.dram_tensor("q_t", [128, L * KC, N], DT_IN, kind="ExternalInput")
    qn_ext = nc.dram_tensor("q_n", [128, 2 * L, D], DT_IN, kind="ExternalInput")
    if HOST_COMBINE:
        lmax_ext = nc.dram_tensor("out_lmax", [128, 2 * L], F32, kind="ExternalOutput")
        invq_ext = nc.dram_tensor("out_invq", [128, 2 * L], F32, kind="ExternalOutput")
        out_ext = cc_in = cc_out = None
    else:
        out_ext = nc.dram_tensor("out", [2, 128], F32, kind="ExternalOutput")
        cc_in = nc.dram_tensor("cc_in", [2 * L, 128], F32)
        cc_out = nc.dram_tensor("cc_out", [2 * L, 128], F32, addr_space="Shared")
        lmax_ext = invq_ext = None

    with tile.TileContext(nc) as tc:
        with (
            tc.tile_pool(name="persist", bufs=1) as pp,
            tc.tile_pool(name="glp", bufs=2) as glpool,
            tc.tile_pool(name="sqp", bufs=BUFS_SQ) as sqpool,
            tc.tile_pool(name="invgp", bufs=4) as invgpool,
            tc.tile_pool(name="simp", bufs=BUFS_SIM) as simpool,
            tc.tile_pool(name="qsqp", bufs=2) as qsqpool,
            tc.tile_pool(name="pmp", bufs=3, space="PSUM") as pmpool,
            tc.tile_pool(name="pnorm", bufs=2, space="PSUM") as pnormpool,
        ):
            # ---- persistent tiles ----
            qt_sb = pp.tile([128, L * KC, N], DT_IN, name="qt_sb")
            qn_sb = pp.tile([128, 2 * L, D], DT_IN, name="qn_sb")
            ones_sb = pp.tile([128, 2, 128], DT_IN, name="ones_sb")
            nc.gpsimd.memset(ones_sb[:], 1.0)
            runmax = pp.tile([128, 2 * L, SUPER], BF16, name="runmax")
            nc.gpsimd.memset(runmax[:], NEG)
            starter = pp.tile([128, KC, STARTER], DT_IN, name="starter")
            qss = pp.tile([128, 2 * L], F32, name="qss")
            invq = pp.tile([128, 2 * L], F32, name="invq")
            lmax_sb = pp.tile([128, 2 * L], F32, name="lmax_sb")
            gmax_sb = pp.tile([128, 2 * L], F32, name="gmax_sb")
            smax_sb = pp.tile([128, 2 * L], F32, name="smax_sb")
            res_sb = pp.tile([128, 2], F32, name="res_sb")
            if KERNEL_TAG:
                tag_sb = pp.tile([128, 1], F32, name=f"tag_{KERNEL_TAG}")
                nc.gpsimd.memset(tag_sb[:], 1.0)

            def emit_q_norm_step(step):
                # spread query-norm work through the main loop (ACT bubbles)
                if step == 0:
                    nc.sync.dma_start(qn_sb[:], qn_ext.ap())
                elif step <= 2 * L:
                    j = step - 1
                    qsq_scr = qsqpool.tile([128, D], BF16, name="qsq_scr")
                    nc.scalar.activation(
                        qsq_scr[:],
                        qn_sb[:, j, :],
                        AF.Square,
                        accum_out=qss[:, j : j + 1],
                    )
                elif step == 2 * L + 1:
                    nc.scalar.activation(invq[:], qss[:], AF.Abs_reciprocal_sqrt)
                    if HOST_COMBINE:
                        nc.sync.dma_start(invq_ext.ap(), invq[:])

            def stage_a(glt, m0, msz, sidx=0):
                # squares: sq[:, k, :msz] = g^2, split ACT / DVE
                sq = sqpool.tile([128, KC, SUPER], DT_IN, name="sq")
                a = SQ_ACT + (1 if SQ_ALT and sidx % 2 else 0)
                if a:
                    nc.scalar.activation(
                        sq[:, :a, :msz], glt[:, :a, m0 : m0 + msz], AF.Square
                    )
                if a < KC:
                    nc.vector.tensor_tensor(
                        out=sq[:, a:, :msz],
                        in0=glt[:, a:, m0 : m0 + msz],
                        in1=glt[:, a:, m0 : m0 + msz],
                        op=mybir.AluOpType.mult,
                    )
                return sq

            def stage_b_pair(items):
                # items: list of (lx, glt, m0, msz, sq), same layer, len 1-2.
                # grouped so consecutive matmuls share stationaries.
                pns = []
                for lx, glt, m0, msz, sq in items:
                    pnorm = pnormpool.tile([128, SUPER], F32, name="pnorm")
                    pns.append(pnorm)
                for j in range(KP):
                    for (lx, glt, m0, msz, sq), pnorm in zip(items, pns):
                        nc.tensor.matmul(
                            pnorm[:, :msz],
                            ones_sb[:],
                            sq[:, 2 * j : 2 * j + 2, :msz],
                            start=(j == 0),
                            stop=(j == KP - 1),
                            perf_mode=DR,
                        )
                invgs = []
                for (lx, glt, m0, msz, sq), pnorm in zip(items, pns):
                    invg = invgpool.tile([128, SUPER], F32, name="invg")
                    nc.scalar.activation(
                        invg[:, :msz], pnorm[:, :msz], AF.Abs_reciprocal_sqrt
                    )
                    invgs.append(invg)

                lx = items[0][0]
                pms = {}
                for i, (lx_, glt, m0, msz, sq) in enumerate(items):
                    # one [128, 2, 512] tile = 2 PSUM banks (cx0 | cx1)
                    pms[i] = pmpool.tile([128, 2, SUPER], F32, name="pmp")
                for cx in range(2):
                    for j in range(KP):
                        for i, (lx_, glt, m0, msz, sq) in enumerate(items):
                            nc.tensor.matmul(
                                pms[i][:, cx, :msz],
                                qt_sb[
                                    :,
                                    lx * KC + 2 * j : lx * KC + 2 * j + 2,
                                    cx * 128 : (cx + 1) * 128,
                                ],
                                glt[:, 2 * j : 2 * j + 2, m0 : m0 + msz],
                                start=(j == 0),
                                stop=(j == KP - 1),
                                perf_mode=DR,
                            )
                for i, (lx_, glt, m0, msz, sq) in enumerate(items):
                    # fused over both cx chunks: one scale-mult, one run-max
                    sim = simpool.tile([128, 2, SUPER], BF16, name="sim")
                    nc.vector.tensor_tensor(
                        out=sim[:, :, :msz],
                        in0=pms[i][:, :, :msz],
                        in1=invgs[i][:, :msz]
                        .unsqueeze(1)
                        .broadcast_to([128, 2, msz]),
                        op=mybir.AluOpType.mult,
                    )
                    nc.vector.tensor_tensor(
                        out=runmax[:, 2 * lx : 2 * lx + 2, :msz],
                        in0=runmax[:, 2 * lx : 2 * lx + 2, :msz],
                        in1=sim[:, :, :msz],
                        op=mybir.AluOpType.max,
                    )

            def layer_done(lx):
                for cx in range(2):
                    j = lx * 2 + cx
                    nc.vector.reduce_max(
                        lmax_sb[:, j : j + 1],
                        runmax[:, j, :],
                        axis=mybir.AxisListType.X,
                    )
                if HOST_COMBINE:
                    nc.sync.dma_start(
                        lmax_ext.ap()[:, 2 * lx : 2 * lx + 2],
                        lmax_sb[:, 2 * lx : 2 * lx + 2],
                    )
                elif SPLIT_CC:
                    nc.sync.dma_start(
                        cc_in.ap()[2 * lx : 2 * lx + 2, :].rearrange("c p -> p c"),
                        lmax_sb[:, 2 * lx : 2 * lx + 2],
                    )
                    nc.gpsimd.collective_compute(
                        "AllReduce",
                        mybir.AluOpType.max,
                        replica_groups=[list(range(NCORES))],
                        ins=[cc_in.ap()[2 * lx : 2 * lx + 2, :].opt()],
                        outs=[cc_out.ap()[2 * lx : 2 * lx + 2, :].opt()],
                    )

            # ---- DMAs: starter chunk, then full contiguous layers ----
            big = {}

            def emit_big_dma(lx):
                m_lo = STARTER if lx == 0 else 0
                w = MS - m_lo
                glt = glpool.tile([128, KC, MS], DT_IN, name="gl")
                if lx == 0:
                    # split so early supers unblock before the full layer lands
                    h = w // 2 // SUPER * SUPER
                    nc.sync.dma_start(
                        glt[:, :, :h], g_ext.ap()[lx][:, :, m_lo : m_lo + h]
                    )
                    nc.sync.dma_start(
                        glt[:, :, h:w], g_ext.ap()[lx][:, :, m_lo + h : MS]
                    )
                else:
                    nc.sync.dma_start(
                        glt[:, :, :w], g_ext.ap()[lx][:, :, m_lo:MS]
                    )
                big[lx] = glt

            nc.sync.dma_start(starter[:], g_ext.ap()[0][:, :, :STARTER])
            nc.sync.dma_start(qt_sb[:], qt_ext.ap())
            emit_q_norm_step(0)  # qn DMA
            emit_big_dma(0)
            emit_big_dma(1)

            # work items: (lx, tile_kind, m0_in_tile, msz)
            def layer_supers(lx):
                out = []
                if lx == 0:
                    for m0 in range(0, STARTER, SUPER):
                        out.append((lx, "S", m0, SUPER))
                    base = STARTER
                else:
                    base = 0
                w = MS - base
                for m0 in range(0, w, SUPER):
                    out.append((lx, "B", m0, min(SUPER, w - m0)))
                return out

            pending = []
            done_lx = 0
            qn_step = 1
            sidx = 0
            for lx in range(L):
                for it in layer_supers(lx):
                    lx_, kind, m0, msz = it
                    if kind == "S":
                        glt = starter
                    else:
                        glt = big[lx_]
                    pending.append((lx_, glt, m0, msz, stage_a(glt, m0, msz, sidx)))
                    if sidx >= 2 and sidx % 2 == 0 and qn_step <= 2 * L + 1:
                        emit_q_norm_step(qn_step)
                        qn_step += 1
                    # pop a pair when we have SKEW_PAIRS*2 + 2 items pending
                    while len(pending) > 2 * SKEW_PAIRS + 1:
                        take = 2 if (
                            len(pending) >= 2 and pending[0][0] == pending[1][0]
                        ) else 1
                        batch, pending = pending[:take], pending[take:]
                        stage_b_pair(batch)
                        nxt = pending[0][0] if pending else L
                        while done_lx < nxt:
                            layer_done(done_lx)
                            done_lx += 1
                    sidx += 1
                if lx + 2 < L:
                    emit_big_dma(lx + 2)
            while pending:
                take = 2 if (
                    len(pending) >= 2 and pending[0][0] == pending[1][0]
                ) else 1
                batch, pending = pending[:take], pending[take:]
                stage_b_pair(batch)
                nxt = pending[0][0] if pending else L
                while done_lx < nxt:
                    layer_done(done_lx)
                    done_lx += 1
            while qn_step <= 2 * L + 1:
                emit_q_norm_step(qn_step)
                qn_step += 1

            if not HOST_COMBINE:
                if not SPLIT_CC:
                    nc.sync.dma_start(
                        cc_in.ap().rearrange("c p -> p c"), lmax_sb[:]
                    )
                    nc.gpsimd.collective_compute(
                        "AllReduce",
                        mybir.AluOpType.max,
                        replica_groups=[list(range(NCORES))],
                        ins=[cc_in.ap().opt()],
                        outs=[cc_out.ap().opt()],
                    )
                nc.sync.dma_start(
                    gmax_sb[:], cc_out.ap().rearrange("c p -> p c")
                )

                # ---- scale by 1/||q||, mean over layers, 1 - x ----
                nc.vector.tensor_tensor(
                    out=smax_sb[:],
                    in0=gmax_sb[:],
                    in1=invq[:],
                    op=mybir.AluOpType.mult,
                )
                for cx in range(2):
                    ssum = pp.tile([128, 1], F32, name=f"ssum{cx}")
                    nc.vector.reduce_sum(
                        ssum[:],
                        smax_sb[:, cx : 2 * L : 2],
                        axis=mybir.AxisListType.X,
                    )
                    nc.scalar.activation(
                        res_sb[:, cx : cx + 1],
                        ssum[:],
                        AF.Copy,
                        bias=1.0,
                        scale=-1.0 / L,
                    )
                for cx in range(2):
                    nc.sync.dma_start(
                        out_ext.ap()[cx : cx + 1, :].rearrange("c p -> p c"),
                        res_sb[:, cx : cx + 1],
                    )

    nc.compile()
    return nc


_NC_CACHE = None


def _get_nc():
    global _NC_CACHE
    if _NC_CACHE is None:
        _patch_ldwopt()
        _NC_CACHE = build()
    return _NC_CACHE


def _prep_shard(g_lp, c):
    # [L, MS, D] slice -> [L, 128, KC, MS] (contiguous partition lines)
    sl = g_lp[:, c * MS : (c + 1) * MS, :]  # [L, MS, D]
    # d = k*128 + p  ->  [L, MS, KC, 128] -> [L, 128, KC, MS]
    return np.ascontiguousarray(
        sl.reshape(L, MS, KC, 128).transpose(0, 3, 2, 1)
    )


def _prep_inputs(test_patch_tokens, normal_patch_tokens):
    q = np.asarray(test_patch_tokens, dtype=np.float32)
    g = np.asarray(normal_patch_tokens, dtype=np.float32)
    qn_lp = q.astype(NP_IN)  # [L, N, D]
    # qn packed [128, 2L, D]: [p, l*2+c, d] = qn[l, c*128+p, d]
    qn_pk = np.ascontiguousarray(
        qn_lp.reshape(L, 2, 128, D).transpose(2, 0, 1, 3).reshape(128, 2 * L, D)
    )
    # qt packed [128, L*KC, N]: [p, l*KC+k, n] = qn[l, n, k*128+p]
    qt_pk = np.ascontiguousarray(
        qn_lp.transpose(0, 2, 1)  # [L, D, N]
        .reshape(L, KC, 128, N)
        .transpose(2, 0, 1, 3)
        .reshape(128, L * KC, N)
    )
    g_lp = g.astype(NP_IN)  # [L, M, D]
    with ThreadPoolExecutor(NCORES) as ex:
        shards = list(ex.map(lambda c: _prep_shard(g_lp, c), range(NCORES)))
    return [
        {"g_t": shards[c], "q_t": qt_pk, "q_n": qn_pk} for c in range(NCORES)
    ]


def kernel(test_patch_tokens: np.ndarray, normal_patch_tokens: np.ndarray):
    in_maps = _prep_inputs(test_patch_tokens, normal_patch_tokens)
    nc = _get_nc()
    results = run_bass_kernel_spmd(nc, in_maps, core_ids=list(range(NCORES))).results
    if HOST_COMBINE:
        # gather/unshard: global max over the 8 gallery shards, then the
        # tiny epilogue (1/||q|| scale, mean over layers, 1-x)
        lmax = np.max(
            np.stack([results[c]["out_lmax"] for c in range(NCORES)]), axis=0
        )  # [128, 2L]: column j = layer*2 + chunk
        invq = results[0]["out_invq"]  # identical on all cores
        smax = lmax * invq
        test_sim = smax.reshape(128, L, 2).mean(axis=1)  # [128(p), 2(chunk)]
        out = 1.0 - test_sim.T.reshape(N)  # n = chunk*128 + p
        return out.astype(np.float32).reshape(1, 1, 16, 16)
    out = results[0]["out"].astype(np.float32).reshape(1, 1, 16, 16)
    return out
